# revision 16
# baseline (speedup 1.0000x reference)
"""Causal self-attention (B=4, T=2048, E=1024, H=16, rope) on 8 trn2 NeuronCores.

Sharding: core c = 2*b + g handles batch b = c//2, head-group g = c%2
(8 of the 16 heads).  Each core:
  - projects its batch's x into q,k (feature-major, rope'd on chip) and v
    for its 8 heads (fp16 matmuls, fp32 accumulate),
  - runs causal attention entirely on-chip (S^T tiles as stationary
    operands, ones-augmented v gives softmax denominators for free),
  - applies a PARTIAL output projection using only its local 512 inner
    dims (Wproj row-shard) -> no collective at all; the host sums the
    two partial [T, E] outputs of each (g=0, g=1) pair.

Scheduling: a single interleaved emission stream.  Attention is
ACT(exp)-bound (~1.1us/step) while its own PE work is ~0.8us/step, so
the next chunk's qkv projection blocks and the previous group's output
projection tiles are woven between attention steps as PE fillers,
keeping the (in-order) PE queue dense.
"""
import sys

for _p in ("/opt/trn_rl_repo", "/root/.axon_site/_ro/trn_rl_repo"):
    if _p not in sys.path:
        sys.path.append(_p)

import numpy as np
from contextlib import ExitStack

import concourse.bass as bass
import concourse.tile as tile
from concourse import bacc, mybir
from concourse.bass_utils import run_bass_kernel_spmd

B, T, E = 4, 2048, 1024
H_TOT, D = 16, 64
HL = 8            # heads per core
F = HL * D        # 512 local q/k/v features
KB = E // 128     # 8 contraction blocks for qkv
TC = T // 512     # 4 time chunks (512 cols)
TT = T // 128     # 16 time tiles
ROPE_THETA = 10000.0

f32 = mybir.dt.float32
f16 = mybir.dt.float16


def build_nc():
    nc = bacc.Bacc(None, target_bir_lowering=False, debug=False)

    xT = nc.declare_dram_parameter("xT", [128, TC, KB, 512], f16, isOutput=False)
    wq = nc.declare_dram_parameter("wq", [128, KB, F], f16, isOutput=False)
    wk = nc.declare_dram_parameter("wk", [128, KB, F], f16, isOutput=False)
    wv = nc.declare_dram_parameter("wv", [128, KB, F], f16, isOutput=False)
    bq = nc.declare_dram_parameter("bq", [128, 4], f32, isOutput=False)  # feature-major cols
    bk = nc.declare_dram_parameter("bk", [128, 4], f32, isOutput=False)
    bvb = nc.declare_dram_parameter("bvb", [128, F], f16, isOutput=False)
    wproj = nc.declare_dram_parameter("wproj", [128, 4, E], f16, isOutput=False)
    bpb = nc.declare_dram_parameter("bpb", [128, E], f32, isOutput=False)
    ctab_d = nc.declare_dram_parameter("ctab", [128, T], f16, isOutput=False)
    stab_d = nc.declare_dram_parameter("stab", [128, T], f16, isOutput=False)
    perm_d = nc.declare_dram_parameter("perm", [128, 128], f16, isOutput=False)
    tri_d = nc.declare_dram_parameter("tri", [128, 128], f16, isOutput=False)  # 0/1 mult mask
    out_ext = nc.declare_dram_parameter("out", [T, E], f16, isOutput=True)

    with ExitStack() as ctx:
        tc = ctx.enter_context(tile.TileContext(nc))
        sres = ctx.enter_context(tc.tile_pool(name="res", bufs=1))
        swts = ctx.enter_context(tc.tile_pool(name="wts", bufs=1))
        stab = ctx.enter_context(tc.tile_pool(name="tab", bufs=1))
        sx = ctx.enter_context(tc.tile_pool(name="x", bufs=2))
        stmp = ctx.enter_context(tc.tile_pool(name="tmp", bufs=2))
        sp = ctx.enter_context(tc.tile_pool(name="p", bufs=3))
        ssm = ctx.enter_context(tc.tile_pool(name="sm", bufs=2))
        sout = ctx.enter_context(tc.tile_pool(name="out", bufs=2))
        pps = ctx.enter_context(tc.tile_pool(name="ps", bufs=2, space="PSUM"))
        ppq = ctx.enter_context(tc.tile_pool(name="pq", bufs=1, space="PSUM"))
        pac = ctx.enter_context(tc.tile_pool(name="ac", bufs=2, space="PSUM"))

        # ---- resident tiles
        qT_t = sres.tile([128, 4, T], f16, tag="qT")       # rope'd q, feature-major
        kT_t = sres.tile([128, 4, T], f16, tag="kT")
        v_t = sres.tile([128, TT, HL, 128], f16, tag="v")  # ones 0:64, v 64:128
        ot_t = sres.tile([128, 4, T], f16, tag="ot")       # attention out, feature-major
        perm_t = sres.tile([128, 128], f16, tag="perm")
        tri_t = sres.tile([128, 128], f16, tag="tri")
        bq_t = sres.tile([128, 4], f32, tag="bq")
        bk_t = sres.tile([128, 4], f32, tag="bk")
        bvb_t = sres.tile([128, F], f16, tag="bvb")
        bpb_t = sres.tile([128, E], f32, tag="bpb")
        ctab_t = stab.tile([128, T], f16, tag="ct")
        stab_t = stab.tile([128, T], f16, tag="st")
        wq_t = swts.tile([128, KB, F], f16, tag="wq")
        wk_t = swts.tile([128, KB, F], f16, tag="wk")
        wv_t = swts.tile([128, KB, F], f16, tag="wv")
        wp_t = swts.tile([128, 4, E], f16, tag="wp")

        # ---- startup DMAs: inputs are host-laid-out so each partition is
        # one contiguous 8KB run (big DMA packets), and the critical loads
        # are split across two queues each for latency.
        nc.gpsimd.dma_start(out=wq_t[:, 0:4, :], in_=wq[:, 0:4, :])
        nc.scalar.dma_start(out=wq_t[:, 4:8, :], in_=wq[:, 4:8, :])
        nc.gpsimd.dma_start(out=perm_t, in_=perm_d[:, :])
        nc.gpsimd.dma_start(out=bq_t, in_=bq[:, :])
        nc.gpsimd.dma_start(out=bk_t, in_=bk[:, :])
        nc.gpsimd.dma_start(out=wk_t[:, 0:4, :], in_=wk[:, 0:4, :])
        nc.scalar.dma_start(out=wk_t[:, 4:8, :], in_=wk[:, 4:8, :])
        nc.gpsimd.dma_start(out=wv_t[:, 0:4, :], in_=wv[:, 0:4, :])
        nc.scalar.dma_start(out=wv_t[:, 4:8, :], in_=wv[:, 4:8, :])
        nc.gpsimd.dma_start(out=bvb_t, in_=bvb[:, :])
        nc.gpsimd.dma_start(out=tri_t, in_=tri_d[:, :])
        nc.gpsimd.dma_start(out=wp_t[:, 0:2, :], in_=wproj[:, 0:2, :])
        nc.scalar.dma_start(out=wp_t[:, 2:4, :], in_=wproj[:, 2:4, :])
        nc.gpsimd.dma_start(out=bpb_t, in_=bpb[:, :])

        x_ts = {}

        def dma_x(tcx, eng2=None):
            # second half goes on the scalar queue at startup (it is idle
            # before the first exp) and on gpsimd mid-run
            eng2 = eng2 or nc.gpsimd
            x_ts[tcx] = sx.tile([128, KB, 512], f16, tag="x", name=f"x{tcx}")
            nc.sync.dma_start(out=x_ts[tcx][:, 0:4, :], in_=xT[:, tcx, 0:4, :])
            eng2.dma_start(out=x_ts[tcx][:, 4:8, :], in_=xT[:, tcx, 4:8, :])

        dma_x(0, eng2=nc.scalar)
        nc.sync.dma_start(out=ctab_t, in_=ctab_d[:, :])
        nc.sync.dma_start(out=stab_t, in_=stab_d[:, :])
        dma_x(1, eng2=nc.scalar)

        # ones columns for the softmax-denominator trick: per-chunk memsets
        # on the (otherwise idle) gpsimd engine so the DVE never stalls on
        # them and chunk-0 v tiles are ready early
        for c in range(TC):
            nc.gpsimd.memset(v_t[:, 4 * c:4 * c + 4, :, 0:D], 1.0)

        # ---- qkv machinery (software-pipelined rope flush)
        st = {"pend": None}

        def flush_pend():
            if st["pend"] is None:
                return
            ps_p, q16, dst, f, cs = st["pend"]
            st["pend"] = None
            nc.tensor.matmul(ps_p, perm_t[:, :], q16[:, :], start=True, stop=True)
            t1 = stmp.tile([128, 512], f16, tag="t1")
            nc.vector.tensor_mul(t1[:, :], q16[:, :], ctab_t[:, cs])
            t2 = stmp.tile([128, 512], f16, tag="t2")
            nc.vector.tensor_mul(t2[:, :], ps_p, stab_t[:, cs])
            nc.vector.tensor_add(dst[:, f, cs], t1[:, :], t2[:, :])

        def qk_block(tcx, which, f, pool=None):
            cs = slice(tcx * 512, (tcx + 1) * 512)
            w_t, b_t, dst = ((wq_t, bq_t, qT_t) if which == "q"
                             else (wk_t, bk_t, kT_t))
            x_t = x_ts[tcx]
            if pool is None:
                ps2 = ppq.tile([128, 1024], f32, tag="qq")
            else:
                ps2 = pool.tile([128, 2, 512], f32, tag="mm", name="ps2b").rearrange(
                    "p a b -> p (a b)")
            ps_q = ps2[:, 0:512]
            ps_p = ps2[:, 512:1024]
            for kb in range(KB):
                nc.tensor.matmul(
                    ps_q,
                    w_t[:, kb, f * 128:(f + 1) * 128],
                    x_t[:, kb, :],
                    start=(kb == 0), stop=(kb == KB - 1),
                )
            q16 = stmp.tile([128, 512], f16, tag="t0")
            nc.vector.tensor_scalar_add(q16[:, :], ps_q, b_t[:, f:f + 1])
            flush_pend()
            st["pend"] = (ps_p, q16, dst, f, cs)

        def v_block(tcx, tl, pool=None):
            tt = tcx * 4 + tl
            x_t = x_ts[tcx]
            if pool is None:
                ps2 = ppq.tile([128, 1024], f32, tag="qq")
            else:
                ps2 = pool.tile([128, 2, 512], f32, tag="mm", name="ps2b").rearrange(
                    "p a b -> p (a b)")
            ps_v = ps2[:, 0:512]
            for kb in range(KB):
                nc.tensor.matmul(
                    ps_v,
                    x_t[:, kb, tl * 128:(tl + 1) * 128],
                    wv_t[:, kb, :],
                    start=(kb == 0), stop=(kb == KB - 1),
                )
            nc.vector.tensor_add(
                v_t[:, tt, :, D:128],
                ps_v.rearrange("p (h d) -> p h d", h=HL),
                bvb_t.rearrange("p (h d) -> p h d", h=HL),
            )

        def qkv_blocks(tcx, alternate=False):
            blocks = ([lambda f=f, p=None: qk_block(tcx, "q", f, p)
                       for f in range(4)]
                      + [lambda f=f, p=None: qk_block(tcx, "k", f, p)
                         for f in range(4)]
                      + [lambda tl=tl, p=None: v_block(tcx, tl, p)
                         for tl in range(4)])
            if alternate:
                # odd blocks borrow the (still idle) attention S pool so the
                # PE never serializes on the single qkv PSUM buffer
                return [lambda b=b, i=i: b(p=pps if i % 2 else None)
                        for i, b in enumerate(blocks)]
            return blocks

        def proj_tile(tt, pool=None):
            flush_pend()
            if pool is None:
                o_ps = ppq.tile([128, 1024], f32, tag="qq")
            else:
                o_ps = pool.tile([128, 2, 512], f32, tag="mm", name="o_psb").rearrange(
                    "p a b -> p (a b)")
            # two accumulation chains, each within one PSUM bank
            for half in range(2):
                for kb in range(4):
                    nc.tensor.matmul(
                        o_ps[:, half * 512:(half + 1) * 512],
                        ot_t[:, kb, tt * 128:(tt + 1) * 128],
                        wp_t[:, kb, half * 512:(half + 1) * 512],
                        start=(kb == 0), stop=(kb == 3),
                    )
            o_st = sout.tile([128, E], f16, tag="o")
            nc.vector.tensor_add(o_st[:, :], o_ps, bpb_t[:, :])
            ts_ = slice(tt * 128, (tt + 1) * 128)
            nc.sync.dma_start(out=out_ext[ts_, 0:512], in_=o_st[:, 0:512])
            nc.gpsimd.dma_start(out=out_ext[ts_, 512:1024], in_=o_st[:, 512:1024])

        # ---- attention
        ps_os = {}

        def emit_S(gi, hp, kt):
            q0 = gi * 512
            kt0 = gi * 4
            j = kt - kt0
            w0 = max(j, 0) * 128
            ncol = 512 - w0
            ps_s = pps.tile([128, 2, 512], f32, tag="mm")
            # one PSUM bank per head-half; exp uses a 2D AP over just the
            # valid cols of both halves (saves ~10% ACT on diagonal steps)
            for half, bp in ((0, 0), (1, 64)):
                nc.tensor.matmul(
                    ps_s[:, half, 0:ncol],
                    kT_t[bp:bp + 64, hp, kt * 128:(kt + 1) * 128],
                    qT_t[bp:bp + 64, hp, q0 + w0:q0 + 512],
                    start=True, stop=True,
                )
            return ps_s, ncol

        def emit_tail(gi, hp, kt, ps_s, ncol):
            q0 = gi * 512
            kt0 = gi * 4
            nkt = kt0 + 4
            j = kt - kt0
            w0 = 512 - ncol
            if kt == 0:
                ps_os[hp] = (pac.tile([128, 512], f32, tag="acc", name="ps_oA"),
                             pac.tile([128, 512], f32, tag="acc", name="ps_oB"))
            p_t = sp.tile([128, 2, 512], f16, tag="p")
            nc.scalar.activation(
                p_t[:, :, 0:ncol], ps_s[:, :, 0:ncol],
                mybir.ActivationFunctionType.Exp, scale=float(D) ** -0.5,
            )
            if j >= 0:
                nc.vector.tensor_mul(p_t[:, 0, 0:128], p_t[:, 0, 0:128],
                                     tri_t[:, :])
                nc.vector.tensor_mul(p_t[:, 1, 0:128], p_t[:, 1, 0:128],
                                     tri_t[:, :])
            for half, h, ps_o in ((0, 2 * hp, ps_os[hp][0]),
                                  (1, 2 * hp + 1, ps_os[hp][1])):
                nc.tensor.matmul(
                    ps_o[:, w0:512],
                    v_t[:, kt, h, :],
                    p_t[:, half, 0:ncol],
                    start=(kt == 0), stop=(kt == nkt - 1),
                )
            if kt == nkt - 1:
                qs = slice(q0, q0 + 512)
                for bp, ps_o in ((0, ps_os[hp][0]), (64, ps_os[hp][1])):
                    recip = ssm.tile([128, 512], f32, tag="rc")
                    nc.vector.reciprocal_approx_fast(out=recip[0:64, 0:512],
                                                     in_=ps_o[0:64, 0:512])
                    nc.vector.tensor_mul(ot_t[bp:bp + 64, hp, qs],
                                         ps_o[64:128, 0:512], recip[0:64, 0:512])
                del ps_os[hp]

        def attn_group(gi, fillers):
            nkt = gi * 4 + 4
            steps = [(hp, kt) for hp in range(HL // 2) for kt in range(nkt)]
            ns = len(steps)
            nf = len(fillers)
            # S emitted one step ahead so the in-order PE always has the
            # next pair's scores in flight while ACT runs the current exp.
            ps_prev = emit_S(gi, *steps[0])
            fi = 0
            for i, (hp, kt) in enumerate(steps):
                ps_cur = ps_prev
                if i + 1 < ns:
                    ps_prev = emit_S(gi, *steps[i + 1])
                emit_tail(gi, hp, kt, *ps_cur)
                # near an hp boundary, hold fillers back so the DVE can run
                # the accumulator-freeing recip/mul chain without backlog
                nkt_ = gi * 4 + 4
                if kt >= nkt_ - 2 and i + 1 < ns:
                    continue
                want = (i + 1) * nf // ns
                while fi < want:
                    fillers[fi]()
                    fi += 1
            while fi < nf:
                fillers[fi]()
                fi += 1

        # ---- emission schedule
        for blk in qkv_blocks(0, alternate=True):
            blk()
        flush_pend()
        attn_group(0, [lambda: dma_x(2)] + qkv_blocks(1))
        flush_pend()
        attn_group(1, [lambda: dma_x(3)] + qkv_blocks(2)
                   + [lambda t=t: proj_tile(t) for t in range(0, 4)])
        flush_pend()
        attn_group(2, qkv_blocks(3))
        flush_pend()
        attn_group(3, [lambda t=t: proj_tile(t) for t in range(4, 12)])
        flush_pend()
        for t in range(12, 16):
            proj_tile(t, pool=pps if t % 2 else None)

    nc.compile()
    return nc


_NC = None


def _get_nc():
    global _NC
    if _NC is None:
        _NC = build_nc()
    return _NC


def _host_prep(x, Wqkv, bqkv, Wproj, bproj):
    """Build the 8 per-core input maps."""
    x = np.asarray(x, np.float32)
    Wqkv = np.asarray(Wqkv, np.float32)
    bqkv = np.asarray(bqkv, np.float32)
    Wproj = np.asarray(Wproj, np.float32)
    bproj = np.asarray(bproj, np.float32)

    perm_d = np.concatenate([np.arange(0, D, 2), np.arange(1, D, 2)])  # evens, odds

    # rope tables (feature-major; rows r: freq r%32, sign -/+ per 32-block)
    inv_freq = 1.0 / ROPE_THETA ** (np.arange(0, D, 2, dtype=np.float32) / D)
    freqs = np.arange(T, dtype=np.float32)[:, None] * inv_freq[None, :]  # (T, 32)
    cosf = np.cos(freqs).T.astype(np.float32)  # (32, T)
    sinf = np.sin(freqs).T.astype(np.float32)
    ctab = np.tile(cosf, (4, 1)).astype(np.float16)                 # (128, T)
    stab = np.concatenate([-sinf, sinf, -sinf, sinf], 0).astype(np.float16)

    # block-swap permutation matrix: out row m <- in row pi(m)
    pmat = np.zeros((128, 128), np.float16)
    for m in range(128):
        base = (m // 64) * 64
        r = m % 64
        pmat[base + (r + 32) % 64, m] = 1.0

    tri = (np.arange(128)[:, None] <= np.arange(128)[None, :]).astype(np.float16)

    maps = []
    for c in range(8):
        b, g = c // 2, c % 2
        heads = np.arange(8 * g, 8 * g + 8)
        # permuted q/k columns, natural v columns
        qcols = (heads[:, None] * D + perm_d[None, :]).ravel()
        vcols = (heads[:, None] * D + np.arange(D)[None, :]).ravel()
        if g == 0:
            bp_full = np.tile(bproj.astype(np.float32), (128, 1))
        else:
            bp_full = np.zeros((128, E), np.float32)
        def relay(w, kb):  # [kb*128, N] -> [128, kb, N] partition-major
            n = w.shape[1]
            return np.ascontiguousarray(
                w.reshape(kb, 128, n).transpose(1, 0, 2).astype(np.float16))

        x3 = np.ascontiguousarray(
            x[b].reshape(TC, 512, KB, 128).transpose(3, 0, 2, 1)
            .astype(np.float16))
        maps.append({
            "xT": x3,
            "wq": relay(Wqkv[:, qcols], KB),
            "wk": relay(Wqkv[:, E + qcols], KB),
            "wv": relay(Wqkv[:, 2 * E + vcols], KB),
            "bq": np.ascontiguousarray(bqkv[qcols].reshape(4, 128).T.astype(np.float32)),
            "bk": np.ascontiguousarray(bqkv[E + qcols].reshape(4, 128).T.astype(np.float32)),
            "bvb": np.ascontiguousarray(
                np.tile(bqkv[2 * E + vcols].astype(np.float16), (128, 1))),
            "wproj": relay(Wproj[vcols, :], 4),
            "bpb": bp_full,
            "ctab": ctab,
            "stab": stab,
            "perm": pmat,
            "tri": tri,
        })
    return maps


def kernel(x, Wqkv, bqkv, Wproj, bproj):
    nc = _get_nc()
    in_maps = _host_prep(x, Wqkv, bqkv, Wproj, bproj)
    res = run_bass_kernel_spmd(nc, in_maps, list(range(8)))
    out = np.empty((B, T, E), np.float32)
    for b in range(B):
        out[b] = (res.results[2 * b]["out"].astype(np.float32)
                  + res.results[2 * b + 1]["out"].astype(np.float32))
    return out


if __name__ == "__main__":
    rng = np.random.default_rng(0)
    x = rng.standard_normal((B, T, E), dtype=np.float32)
    Wqkv = rng.standard_normal((E, 3 * E), dtype=np.float32) * 0.02
    bqkv = rng.standard_normal((3 * E,), dtype=np.float32) * 0.02
    Wproj = rng.standard_normal((E, E), dtype=np.float32) * 0.02
    bproj = rng.standard_normal((E,), dtype=np.float32) * 0.02
    o = kernel(x=x, Wqkv=Wqkv, bqkv=bqkv, Wproj=Wproj, bproj=bproj)
    print("out", o.shape, o.dtype, float(np.abs(o).max()))


# revision 18
# speedup vs baseline: 1.0168x; 1.0168x over previous
"""Causal self-attention (B=4, T=2048, E=1024, H=16, rope) on 8 trn2 NeuronCores.

Sharding: core c = 2*b + g handles batch b = c//2, head-group g = c%2
(8 of the 16 heads).  Each core:
  - projects its batch's x into q,k (feature-major, rope'd on chip) and v
    for its 8 heads (fp16 matmuls, fp32 accumulate),
  - runs causal attention entirely on-chip (S^T tiles as stationary
    operands, ones-augmented v gives softmax denominators for free),
  - applies a PARTIAL output projection using only its local 512 inner
    dims (Wproj row-shard) -> no collective at all; the host sums the
    two partial [T, E] outputs of each (g=0, g=1) pair.

Scheduling: a single interleaved emission stream.  Attention is
ACT(exp)-bound (~1.1us/step) while its own PE work is ~0.8us/step, so
the next chunk's qkv projection blocks and the previous group's output
projection tiles are woven between attention steps as PE fillers,
keeping the (in-order) PE queue dense.
"""
import sys

for _p in ("/opt/trn_rl_repo", "/root/.axon_site/_ro/trn_rl_repo"):
    if _p not in sys.path:
        sys.path.append(_p)

import numpy as np
from contextlib import ExitStack

import concourse.bass as bass
import concourse.tile as tile
from concourse import bacc, mybir
from concourse.bass_utils import run_bass_kernel_spmd

B, T, E = 4, 2048, 1024
H_TOT, D = 16, 64
HL = 8            # heads per core
F = HL * D        # 512 local q/k/v features
KB = E // 128     # 8 contraction blocks for qkv
TC = T // 512     # 4 time chunks (512 cols)
TT = T // 128     # 16 time tiles
ROPE_THETA = 10000.0

f32 = mybir.dt.float32
f16 = mybir.dt.float16


def build_nc():
    nc = bacc.Bacc(None, target_bir_lowering=False, debug=False)

    xT = nc.declare_dram_parameter("xT", [128, TC, KB, 512], f16, isOutput=False)
    wq = nc.declare_dram_parameter("wq", [128, 4, KB, 128], f16, isOutput=False)
    wk = nc.declare_dram_parameter("wk", [128, 4, KB, 128], f16, isOutput=False)
    wv = nc.declare_dram_parameter("wv", [128, KB, F], f16, isOutput=False)
    bq = nc.declare_dram_parameter("bq", [128, 4], f32, isOutput=False)  # feature-major cols
    bk = nc.declare_dram_parameter("bk", [128, 4], f32, isOutput=False)
    bvb = nc.declare_dram_parameter("bvb", [128, F], f16, isOutput=False)
    wproj = nc.declare_dram_parameter("wproj", [128, 4, E], f16, isOutput=False)
    bpb = nc.declare_dram_parameter("bpb", [128, E], f32, isOutput=False)
    ctab_d = nc.declare_dram_parameter("ctab", [128, T], f16, isOutput=False)
    stab_d = nc.declare_dram_parameter("stab", [128, T], f16, isOutput=False)
    perm_d = nc.declare_dram_parameter("perm", [128, 128], f16, isOutput=False)
    tri_d = nc.declare_dram_parameter("tri", [128, 128], f16, isOutput=False)  # 0/1 mult mask
    out_ext = nc.declare_dram_parameter("out", [T, E], f16, isOutput=True)

    with ExitStack() as ctx:
        tc = ctx.enter_context(tile.TileContext(nc))
        sres = ctx.enter_context(tc.tile_pool(name="res", bufs=1))
        swts = ctx.enter_context(tc.tile_pool(name="wts", bufs=1))
        stab = ctx.enter_context(tc.tile_pool(name="tab", bufs=1))
        sx = ctx.enter_context(tc.tile_pool(name="x", bufs=2))
        stmp = ctx.enter_context(tc.tile_pool(name="tmp", bufs=2))
        sp = ctx.enter_context(tc.tile_pool(name="p", bufs=3))
        ssm = ctx.enter_context(tc.tile_pool(name="sm", bufs=2))
        sout = ctx.enter_context(tc.tile_pool(name="out", bufs=2))
        pps = ctx.enter_context(tc.tile_pool(name="ps", bufs=2, space="PSUM"))
        ppq = ctx.enter_context(tc.tile_pool(name="pq", bufs=1, space="PSUM"))
        pac = ctx.enter_context(tc.tile_pool(name="ac", bufs=2, space="PSUM"))

        # ---- resident tiles
        qT_t = sres.tile([128, 4, T], f16, tag="qT")       # rope'd q, feature-major
        kT_t = sres.tile([128, 4, T], f16, tag="kT")
        v_t = sres.tile([128, TT, HL, 128], f16, tag="v")  # ones 0:64, v 64:128
        ot_t = sres.tile([128, 4, T], f16, tag="ot")       # attention out, feature-major
        perm_t = sres.tile([128, 128], f16, tag="perm")
        tri_t = sres.tile([128, 128], f16, tag="tri")
        bq_t = sres.tile([128, 4], f32, tag="bq")
        bk_t = sres.tile([128, 4], f32, tag="bk")
        bvb_t = sres.tile([128, F], f16, tag="bvb")
        bpb_t = sres.tile([128, E], f32, tag="bpb")
        ctab_t = stab.tile([128, T], f16, tag="ct")
        stab_t = stab.tile([128, T], f16, tag="st")
        wq_t = swts.tile([128, 4, KB, 128], f16, tag="wq")
        wk_t = swts.tile([128, 4, KB, 128], f16, tag="wk")
        wv_t = swts.tile([128, KB, F], f16, tag="wv")
        wp_t = swts.tile([128, 4, E], f16, tag="wp")

        # ---- startup DMAs: inputs are host-laid-out so each partition is
        # one contiguous run (big DMA packets).  Each queue sustains only
        # ~125GB/s and transfers start ~8us in, so the first-needed data
        # (x chunk 0, wq f-strips) is split 3 ways / strip-wise and
        # priority-ordered so the first matmul can issue ~12us.
        x_ts = {}
        x_ts[0] = sx.tile([128, KB, 512], f16, tag="x", name="x0")
        nc.sync.dma_start(out=x_ts[0][:, 0:3, :], in_=xT[:, 0, 0:3, :])
        nc.scalar.dma_start(out=x_ts[0][:, 3:6, :], in_=xT[:, 0, 3:6, :])
        nc.gpsimd.dma_start(out=wq_t[:, 0], in_=wq[:, 0])
        nc.gpsimd.dma_start(out=x_ts[0][:, 6:8, :], in_=xT[:, 0, 6:8, :])
        nc.gpsimd.dma_start(out=wq_t[:, 1], in_=wq[:, 1])
        nc.scalar.dma_start(out=wq_t[:, 2], in_=wq[:, 2])
        nc.scalar.dma_start(out=wq_t[:, 3], in_=wq[:, 3])
        nc.gpsimd.dma_start(out=perm_t, in_=perm_d[:, :])
        nc.gpsimd.dma_start(out=bq_t, in_=bq[:, :])
        nc.gpsimd.dma_start(out=bk_t, in_=bk[:, :])
        nc.sync.dma_start(out=ctab_t, in_=ctab_d[:, :])
        nc.sync.dma_start(out=stab_t, in_=stab_d[:, :])
        nc.scalar.dma_start(out=wk_t[:, 0], in_=wk[:, 0])
        nc.scalar.dma_start(out=wk_t[:, 1], in_=wk[:, 1])
        nc.gpsimd.dma_start(out=wv_t[:, 0:4, :], in_=wv[:, 0:4, :])
        nc.scalar.dma_start(out=wk_t[:, 2], in_=wk[:, 2])
        nc.scalar.dma_start(out=wk_t[:, 3], in_=wk[:, 3])
        nc.scalar.dma_start(out=wv_t[:, 4:8, :], in_=wv[:, 4:8, :])
        nc.gpsimd.dma_start(out=bvb_t, in_=bvb[:, :])
        nc.gpsimd.dma_start(out=tri_t, in_=tri_d[:, :])
        # ones columns for the softmax-denominator trick, on gpsimd so the
        # DVE never waits; chunk-0 quarter first
        nc.gpsimd.memset(v_t[:, 0:4, :, 0:D], 1.0)
        nc.gpsimd.memset(v_t[:, 4:8, :, 0:D], 1.0)
        nc.gpsimd.dma_start(out=wp_t[:, 0:2, :], in_=wproj[:, 0:2, :])
        nc.scalar.dma_start(out=wp_t[:, 2:4, :], in_=wproj[:, 2:4, :])
        nc.gpsimd.dma_start(out=bpb_t, in_=bpb[:, :])
        nc.gpsimd.memset(v_t[:, 8:12, :, 0:D], 1.0)
        nc.gpsimd.memset(v_t[:, 12:16, :, 0:D], 1.0)

        def dma_x(tcx, eng2=None):
            eng2 = eng2 or nc.gpsimd
            x_ts[tcx] = sx.tile([128, KB, 512], f16, tag="x", name=f"x{tcx}")
            nc.sync.dma_start(out=x_ts[tcx][:, 0:4, :], in_=xT[:, tcx, 0:4, :])
            eng2.dma_start(out=x_ts[tcx][:, 4:8, :], in_=xT[:, tcx, 4:8, :])

        dma_x(1, eng2=nc.sync)

        # ---- qkv machinery (software-pipelined rope flush)
        st = {"pend": None}

        def flush_pend():
            if st["pend"] is None:
                return
            ps_p, q16, dst, f, cs = st["pend"]
            st["pend"] = None
            nc.tensor.matmul(ps_p, perm_t[:, :], q16[:, :], start=True, stop=True)
            t1 = stmp.tile([128, 512], f16, tag="t1")
            nc.vector.tensor_mul(t1[:, :], q16[:, :], ctab_t[:, cs])
            t2 = stmp.tile([128, 512], f16, tag="t2")
            nc.vector.tensor_mul(t2[:, :], ps_p, stab_t[:, cs])
            nc.vector.tensor_add(dst[:, f, cs], t1[:, :], t2[:, :])

        def qk_block(tcx, which, f, pool=None):
            cs = slice(tcx * 512, (tcx + 1) * 512)
            w_t, b_t, dst = ((wq_t, bq_t, qT_t) if which == "q"
                             else (wk_t, bk_t, kT_t))
            x_t = x_ts[tcx]
            if pool is None:
                ps2 = ppq.tile([128, 1024], f32, tag="qq")
            else:
                ps2 = pool.tile([128, 2, 512], f32, tag="mm", name="ps2b").rearrange(
                    "p a b -> p (a b)")
            ps_q = ps2[:, 0:512]
            ps_p = ps2[:, 512:1024]
            for kb in range(KB):
                nc.tensor.matmul(
                    ps_q,
                    w_t[:, f, kb, :],
                    x_t[:, kb, :],
                    start=(kb == 0), stop=(kb == KB - 1),
                )
            q16 = stmp.tile([128, 512], f16, tag="t0")
            nc.vector.tensor_scalar_add(q16[:, :], ps_q, b_t[:, f:f + 1])
            flush_pend()
            st["pend"] = (ps_p, q16, dst, f, cs)

        def v_block(tcx, tl, pool=None):
            tt = tcx * 4 + tl
            x_t = x_ts[tcx]
            if pool is None:
                ps2 = ppq.tile([128, 1024], f32, tag="qq")
            else:
                ps2 = pool.tile([128, 2, 512], f32, tag="mm", name="ps2b").rearrange(
                    "p a b -> p (a b)")
            ps_v = ps2[:, 0:512]
            for kb in range(KB):
                nc.tensor.matmul(
                    ps_v,
                    x_t[:, kb, tl * 128:(tl + 1) * 128],
                    wv_t[:, kb, :],
                    start=(kb == 0), stop=(kb == KB - 1),
                )
            nc.vector.tensor_add(
                v_t[:, tt, :, D:128],
                ps_v.rearrange("p (h d) -> p h d", h=HL),
                bvb_t.rearrange("p (h d) -> p h d", h=HL),
            )

        def qkv_blocks(tcx, alternate=False):
            blocks = ([lambda f=f, p=None: qk_block(tcx, "q", f, p)
                       for f in range(4)]
                      + [lambda f=f, p=None: qk_block(tcx, "k", f, p)
                         for f in range(4)]
                      + [lambda tl=tl, p=None: v_block(tcx, tl, p)
                         for tl in range(4)])
            if alternate:
                # odd blocks borrow the (still idle) attention S pool so the
                # PE never serializes on the single qkv PSUM buffer
                return [lambda b=b, i=i: b(p=pps if i % 2 else None)
                        for i, b in enumerate(blocks)]
            return blocks

        def proj_tile(tt, pool=None):
            flush_pend()
            if pool is None:
                o_ps = ppq.tile([128, 1024], f32, tag="qq")
            else:
                o_ps = pool.tile([128, 2, 512], f32, tag="mm", name="o_psb").rearrange(
                    "p a b -> p (a b)")
            # two accumulation chains, each within one PSUM bank
            for half in range(2):
                for kb in range(4):
                    nc.tensor.matmul(
                        o_ps[:, half * 512:(half + 1) * 512],
                        ot_t[:, kb, tt * 128:(tt + 1) * 128],
                        wp_t[:, kb, half * 512:(half + 1) * 512],
                        start=(kb == 0), stop=(kb == 3),
                    )
            o_st = sout.tile([128, E], f16, tag="o")
            ts_ = slice(tt * 128, (tt + 1) * 128)
            for half, eng in ((0, nc.sync), (1, nc.gpsimd)):
                hs = slice(half * 512, (half + 1) * 512)
                nc.vector.tensor_add(o_st[:, hs], o_ps[:, hs], bpb_t[:, hs])
                eng.dma_start(out=out_ext[ts_, hs], in_=o_st[:, hs])

        # ---- attention
        ps_os = {}

        def emit_S(gi, hp, kt):
            q0 = gi * 512
            kt0 = gi * 4
            j = kt - kt0
            w0 = max(j, 0) * 128
            ncol = 512 - w0
            ps_s = pps.tile([128, 2, 512], f32, tag="mm")
            # one PSUM bank per head-half; exp uses a 2D AP over just the
            # valid cols of both halves (saves ~10% ACT on diagonal steps)
            for half, bp in ((0, 0), (1, 64)):
                nc.tensor.matmul(
                    ps_s[:, half, 0:ncol],
                    kT_t[bp:bp + 64, hp, kt * 128:(kt + 1) * 128],
                    qT_t[bp:bp + 64, hp, q0 + w0:q0 + 512],
                    start=True, stop=True,
                )
            return ps_s, ncol

        def emit_tail(gi, hp, kt, ps_s, ncol):
            q0 = gi * 512
            kt0 = gi * 4
            nkt = kt0 + 4
            j = kt - kt0
            w0 = 512 - ncol
            if kt == 0:
                ps_os[hp] = (pac.tile([128, 512], f32, tag="acc", name="ps_oA"),
                             pac.tile([128, 512], f32, tag="acc", name="ps_oB"))
            p_t = sp.tile([128, 2, 512], f16, tag="p")
            nc.scalar.activation(
                p_t[:, :, 0:ncol], ps_s[:, :, 0:ncol],
                mybir.ActivationFunctionType.Exp, scale=float(D) ** -0.5,
            )
            if j >= 0:
                nc.vector.tensor_mul(p_t[:, 0, 0:128], p_t[:, 0, 0:128],
                                     tri_t[:, :])
                nc.vector.tensor_mul(p_t[:, 1, 0:128], p_t[:, 1, 0:128],
                                     tri_t[:, :])
            for half, h, ps_o in ((0, 2 * hp, ps_os[hp][0]),
                                  (1, 2 * hp + 1, ps_os[hp][1])):
                nc.tensor.matmul(
                    ps_o[:, w0:512],
                    v_t[:, kt, h, :],
                    p_t[:, half, 0:ncol],
                    start=(kt == 0), stop=(kt == nkt - 1),
                )
            if kt == nkt - 1:
                qs = slice(q0, q0 + 512)
                for bp, ps_o in ((0, ps_os[hp][0]), (64, ps_os[hp][1])):
                    recip = ssm.tile([128, 512], f32, tag="rc")
                    nc.vector.reciprocal_approx_fast(out=recip[0:64, 0:512],
                                                     in_=ps_o[0:64, 0:512])
                    nc.vector.tensor_mul(ot_t[bp:bp + 64, hp, qs],
                                         ps_o[64:128, 0:512], recip[0:64, 0:512])
                del ps_os[hp]

        def attn_group(gi, fillers):
            nkt = gi * 4 + 4
            steps = [(hp, kt) for hp in range(HL // 2) for kt in range(nkt)]
            ns = len(steps)
            nf = len(fillers)
            # S emitted one step ahead so the in-order PE always has the
            # next pair's scores in flight while ACT runs the current exp.
            ps_prev = emit_S(gi, *steps[0])
            fi = 0
            for i, (hp, kt) in enumerate(steps):
                ps_cur = ps_prev
                if i + 1 < ns:
                    ps_prev = emit_S(gi, *steps[i + 1])
                emit_tail(gi, hp, kt, *ps_cur)
                # near an hp boundary, hold fillers back so the DVE can run
                # the accumulator-freeing recip/mul chain without backlog
                nkt_ = gi * 4 + 4
                if kt >= nkt_ - 2 and i + 1 < ns:
                    continue
                want = (i + 1) * nf // ns
                while fi < want:
                    fillers[fi]()
                    fi += 1
            while fi < nf:
                fillers[fi]()
                fi += 1

        # ---- emission schedule
        for blk in qkv_blocks(0, alternate=True):
            blk()
        flush_pend()
        attn_group(0, [lambda: dma_x(2)] + qkv_blocks(1))
        flush_pend()
        attn_group(1, [lambda: dma_x(3)] + qkv_blocks(2)
                   + [lambda t=t: proj_tile(t) for t in range(0, 4)])
        flush_pend()
        attn_group(2, qkv_blocks(3))
        flush_pend()
        attn_group(3, [lambda t=t: proj_tile(t) for t in range(4, 12)])
        flush_pend()
        for t in range(12, 16):
            proj_tile(t, pool=pps if t % 2 else None)

    nc.compile()
    return nc


_NC = None


def _get_nc():
    global _NC
    if _NC is None:
        _NC = build_nc()
    return _NC


def _host_prep(x, Wqkv, bqkv, Wproj, bproj):
    """Build the 8 per-core input maps."""
    x = np.asarray(x, np.float32)
    Wqkv = np.asarray(Wqkv, np.float32)
    bqkv = np.asarray(bqkv, np.float32)
    Wproj = np.asarray(Wproj, np.float32)
    bproj = np.asarray(bproj, np.float32)

    perm_d = np.concatenate([np.arange(0, D, 2), np.arange(1, D, 2)])  # evens, odds

    # rope tables (feature-major; rows r: freq r%32, sign -/+ per 32-block)
    inv_freq = 1.0 / ROPE_THETA ** (np.arange(0, D, 2, dtype=np.float32) / D)
    freqs = np.arange(T, dtype=np.float32)[:, None] * inv_freq[None, :]  # (T, 32)
    cosf = np.cos(freqs).T.astype(np.float32)  # (32, T)
    sinf = np.sin(freqs).T.astype(np.float32)
    ctab = np.tile(cosf, (4, 1)).astype(np.float16)                 # (128, T)
    stab = np.concatenate([-sinf, sinf, -sinf, sinf], 0).astype(np.float16)

    # block-swap permutation matrix: out row m <- in row pi(m)
    pmat = np.zeros((128, 128), np.float16)
    for m in range(128):
        base = (m // 64) * 64
        r = m % 64
        pmat[base + (r + 32) % 64, m] = 1.0

    tri = (np.arange(128)[:, None] <= np.arange(128)[None, :]).astype(np.float16)

    maps = []
    for c in range(8):
        b, g = c // 2, c % 2
        heads = np.arange(8 * g, 8 * g + 8)
        # permuted q/k columns, natural v columns
        qcols = (heads[:, None] * D + perm_d[None, :]).ravel()
        vcols = (heads[:, None] * D + np.arange(D)[None, :]).ravel()
        if g == 0:
            bp_full = np.tile(bproj.astype(np.float32), (128, 1))
        else:
            bp_full = np.zeros((128, E), np.float32)
        def relay(w, kb):  # [kb*128, N] -> [128, kb, N] partition-major
            n = w.shape[1]
            return np.ascontiguousarray(
                w.reshape(kb, 128, n).transpose(1, 0, 2).astype(np.float16))

        def relay_f(w):  # [KB*128, 4*128] -> [128, 4, KB, 128] f-strip major
            return np.ascontiguousarray(
                w.reshape(KB, 128, 4, 128).transpose(1, 2, 0, 3)
                .astype(np.float16))

        x3 = np.ascontiguousarray(
            x[b].reshape(TC, 512, KB, 128).transpose(3, 0, 2, 1)
            .astype(np.float16))
        maps.append({
            "xT": x3,
            "wq": relay_f(Wqkv[:, qcols]),
            "wk": relay_f(Wqkv[:, E + qcols]),
            "wv": relay(Wqkv[:, 2 * E + vcols], KB),
            "bq": np.ascontiguousarray(bqkv[qcols].reshape(4, 128).T.astype(np.float32)),
            "bk": np.ascontiguousarray(bqkv[E + qcols].reshape(4, 128).T.astype(np.float32)),
            "bvb": np.ascontiguousarray(
                np.tile(bqkv[2 * E + vcols].astype(np.float16), (128, 1))),
            "wproj": relay(Wproj[vcols, :], 4),
            "bpb": bp_full,
            "ctab": ctab,
            "stab": stab,
            "perm": pmat,
            "tri": tri,
        })
    return maps


def kernel(x, Wqkv, bqkv, Wproj, bproj):
    nc = _get_nc()
    in_maps = _host_prep(x, Wqkv, bqkv, Wproj, bproj)
    res = run_bass_kernel_spmd(nc, in_maps, list(range(8)))
    out = np.empty((B, T, E), np.float32)
    for b in range(B):
        out[b] = (res.results[2 * b]["out"].astype(np.float32)
                  + res.results[2 * b + 1]["out"].astype(np.float32))
    return out


if __name__ == "__main__":
    rng = np.random.default_rng(0)
    x = rng.standard_normal((B, T, E), dtype=np.float32)
    Wqkv = rng.standard_normal((E, 3 * E), dtype=np.float32) * 0.02
    bqkv = rng.standard_normal((3 * E,), dtype=np.float32) * 0.02
    Wproj = rng.standard_normal((E, E), dtype=np.float32) * 0.02
    bproj = rng.standard_normal((E,), dtype=np.float32) * 0.02
    o = kernel(x=x, Wqkv=Wqkv, bqkv=bqkv, Wproj=Wproj, bproj=bproj)
    print("out", o.shape, o.dtype, float(np.abs(o).max()))


# revision 20
# speedup vs baseline: 1.0248x; 1.0078x over previous
"""Causal self-attention (B=4, T=2048, E=1024, H=16, rope) on 8 trn2 NeuronCores.

Sharding: core c = 2*b + g handles batch b = c//2, head-group g = c%2
(8 of the 16 heads).  Each core:
  - projects its batch's x into q,k (feature-major, rope'd on chip) and v
    for its 8 heads (fp16 matmuls, fp32 accumulate),
  - runs causal attention entirely on-chip (S^T tiles as stationary
    operands, ones-augmented v gives softmax denominators for free),
  - applies a PARTIAL output projection using only its local 512 inner
    dims (Wproj row-shard) -> no collective at all; the host sums the
    two partial [T, E] outputs of each (g=0, g=1) pair.

Scheduling: a single interleaved emission stream.  Attention is
ACT(exp)-bound (~1.1us/step) while its own PE work is ~0.8us/step, so
the next chunk's qkv projection blocks and the previous group's output
projection tiles are woven between attention steps as PE fillers,
keeping the (in-order) PE queue dense.
"""
import sys

for _p in ("/opt/trn_rl_repo", "/root/.axon_site/_ro/trn_rl_repo"):
    if _p not in sys.path:
        sys.path.append(_p)

import numpy as np
from contextlib import ExitStack

import concourse.bass as bass
import concourse.tile as tile
from concourse import bacc, mybir
from concourse.bass_utils import run_bass_kernel_spmd

B, T, E = 4, 2048, 1024
H_TOT, D = 16, 64
HL = 8            # heads per core
F = HL * D        # 512 local q/k/v features
KB = E // 128     # 8 contraction blocks for qkv
TC = T // 512     # 4 time chunks (512 cols)
TT = T // 128     # 16 time tiles
ROPE_THETA = 10000.0

f32 = mybir.dt.float32
f16 = mybir.dt.float16


def build_nc():
    nc = bacc.Bacc(None, target_bir_lowering=False, debug=False)

    xT = nc.declare_dram_parameter("xT", [128, TC, KB, 512], f16, isOutput=False)
    wq = nc.declare_dram_parameter("wq", [128, 4, KB, 128], f16, isOutput=False)
    wk = nc.declare_dram_parameter("wk", [128, 4, KB, 128], f16, isOutput=False)
    wv = nc.declare_dram_parameter("wv", [128, KB, F], f16, isOutput=False)
    bq = nc.declare_dram_parameter("bq", [128, 4], f32, isOutput=False)  # feature-major cols
    bk = nc.declare_dram_parameter("bk", [128, 4], f32, isOutput=False)
    bvb = nc.declare_dram_parameter("bvb", [128, F], f16, isOutput=False)
    wproj = nc.declare_dram_parameter("wproj", [128, 4, E], f16, isOutput=False)
    bpb = nc.declare_dram_parameter("bpb", [128, E], f32, isOutput=False)
    ctab_d = nc.declare_dram_parameter("ctab", [128, T], f16, isOutput=False)
    stab_d = nc.declare_dram_parameter("stab", [128, T], f16, isOutput=False)
    tri_d = nc.declare_dram_parameter("tri", [128, 128], f16, isOutput=False)  # 0/1 mult mask
    out_ext = nc.declare_dram_parameter("out", [T, E], f16, isOutput=True)

    with ExitStack() as ctx:
        tc = ctx.enter_context(tile.TileContext(nc))
        sres = ctx.enter_context(tc.tile_pool(name="res", bufs=1))
        swts = ctx.enter_context(tc.tile_pool(name="wts", bufs=1))
        stab = ctx.enter_context(tc.tile_pool(name="tab", bufs=1))
        sx = ctx.enter_context(tc.tile_pool(name="x", bufs=2))
        stmp = ctx.enter_context(tc.tile_pool(name="tmp", bufs=2))
        sp = ctx.enter_context(tc.tile_pool(name="p", bufs=3))
        ssm = ctx.enter_context(tc.tile_pool(name="sm", bufs=2))
        sout = ctx.enter_context(tc.tile_pool(name="out", bufs=2))
        pps = ctx.enter_context(tc.tile_pool(name="ps", bufs=2, space="PSUM"))
        ppq = ctx.enter_context(tc.tile_pool(name="pq", bufs=1, space="PSUM"))
        pac = ctx.enter_context(tc.tile_pool(name="ac", bufs=2, space="PSUM"))

        # ---- resident tiles
        qT_t = sres.tile([128, 4, T], f16, tag="qT")       # rope'd q, feature-major
        kT_t = sres.tile([128, 4, T], f16, tag="kT")
        v_t = sres.tile([128, TT, HL, 128], f16, tag="v")  # ones 0:64, v 64:128
        ot_t = sres.tile([128, 4, T], f16, tag="ot")       # attention out, feature-major
        tri_t = sres.tile([128, 128], f16, tag="tri")
        bq_t = sres.tile([128, 4], f32, tag="bq")
        bk_t = sres.tile([128, 4], f32, tag="bk")
        bvb_t = sres.tile([128, F], f16, tag="bvb")
        bpb_t = sres.tile([128, E], f32, tag="bpb")
        ctab_t = stab.tile([128, T], f16, tag="ct")
        stab_t = stab.tile([128, T], f16, tag="st")
        wq_t = swts.tile([128, 4, KB, 128], f16, tag="wq")
        wk_t = swts.tile([128, 4, KB, 128], f16, tag="wk")
        wv_t = swts.tile([128, KB, F], f16, tag="wv")
        wp_t = swts.tile([128, 4, E], f16, tag="wp")

        # ---- startup DMAs: inputs are host-laid-out so each partition is
        # one contiguous run (big DMA packets).  Each queue sustains only
        # ~125GB/s and transfers start ~8us in, so the first-needed data
        # (x chunk 0, wq f-strips) is split 3 ways / strip-wise and
        # priority-ordered so the first matmul can issue ~12us.
        x_ts = {}
        x_ts[0] = sx.tile([128, KB, 512], f16, tag="x", name="x0")
        nc.sync.dma_start(out=x_ts[0][:, 0:3, :], in_=xT[:, 0, 0:3, :])
        nc.scalar.dma_start(out=x_ts[0][:, 3:6, :], in_=xT[:, 0, 3:6, :])
        nc.gpsimd.dma_start(out=wq_t[:, 0], in_=wq[:, 0])
        nc.gpsimd.dma_start(out=x_ts[0][:, 6:8, :], in_=xT[:, 0, 6:8, :])
        nc.gpsimd.dma_start(out=wq_t[:, 1], in_=wq[:, 1])
        nc.scalar.dma_start(out=wq_t[:, 2], in_=wq[:, 2])
        nc.scalar.dma_start(out=wq_t[:, 3], in_=wq[:, 3])
        nc.gpsimd.dma_start(out=bq_t, in_=bq[:, :])
        nc.gpsimd.dma_start(out=bk_t, in_=bk[:, :])
        nc.sync.dma_start(out=ctab_t, in_=ctab_d[:, :])
        nc.sync.dma_start(out=stab_t, in_=stab_d[:, :])
        nc.scalar.dma_start(out=wk_t[:, 0], in_=wk[:, 0])
        nc.scalar.dma_start(out=wk_t[:, 1], in_=wk[:, 1])
        nc.gpsimd.dma_start(out=wv_t[:, 0:4, :], in_=wv[:, 0:4, :])
        nc.scalar.dma_start(out=wk_t[:, 2], in_=wk[:, 2])
        nc.scalar.dma_start(out=wk_t[:, 3], in_=wk[:, 3])
        nc.scalar.dma_start(out=wv_t[:, 4:8, :], in_=wv[:, 4:8, :])
        nc.gpsimd.dma_start(out=bvb_t, in_=bvb[:, :])
        nc.gpsimd.dma_start(out=tri_t, in_=tri_d[:, :])
        # ones columns for the softmax-denominator trick, on gpsimd so the
        # DVE never waits; chunk-0 quarter first
        nc.gpsimd.memset(v_t[:, 0:4, :, 0:D], 1.0)
        nc.gpsimd.memset(v_t[:, 4:8, :, 0:D], 1.0)
        nc.gpsimd.dma_start(out=wp_t[:, 0:2, :], in_=wproj[:, 0:2, :])
        nc.scalar.dma_start(out=wp_t[:, 2:4, :], in_=wproj[:, 2:4, :])
        nc.gpsimd.dma_start(out=bpb_t, in_=bpb[:, :])
        nc.gpsimd.memset(v_t[:, 8:12, :, 0:D], 1.0)
        nc.gpsimd.memset(v_t[:, 12:16, :, 0:D], 1.0)

        def dma_x(tcx, eng2=None):
            eng2 = eng2 or nc.gpsimd
            x_ts[tcx] = sx.tile([128, KB, 512], f16, tag="x", name=f"x{tcx}")
            nc.sync.dma_start(out=x_ts[tcx][:, 0:4, :], in_=xT[:, tcx, 0:4, :])
            eng2.dma_start(out=x_ts[tcx][:, 4:8, :], in_=xT[:, tcx, 4:8, :])

        dma_x(1, eng2=nc.sync)

        # ---- qkv machinery (software-pipelined rope flush)
        st = {"pend": None}

        def flush_pend():
            if st["pend"] is None:
                return
            qp, q16, dst, f, cs = st["pend"]
            st["pend"] = None
            t1 = stmp.tile([128, 512], f16, tag="t1")
            nc.vector.tensor_mul(t1[:, :], q16[:, :], ctab_t[:, cs])
            t2 = stmp.tile([128, 512], f16, tag="t2")
            nc.vector.tensor_mul(t2[:, :], qp[:, :], stab_t[:, cs])
            nc.vector.tensor_add(dst[:, f, cs], t1[:, :], t2[:, :])

        def qk_block(tcx, which, f, pool=None):
            cs = slice(tcx * 512, (tcx + 1) * 512)
            w_t, b_t, dst = ((wq_t, bq_t, qT_t) if which == "q"
                             else (wk_t, bk_t, kT_t))
            x_t = x_ts[tcx]
            if pool is None:
                ps2 = ppq.tile([128, 1024], f32, tag="qq")
            else:
                ps2 = pool.tile([128, 2, 512], f32, tag="mm", name="ps2b").rearrange(
                    "p a b -> p (a b)")
            ps_q = ps2[:, 0:512]
            for kb in range(KB):
                nc.tensor.matmul(
                    ps_q,
                    w_t[:, f, kb, :],
                    x_t[:, kb, :],
                    start=(kb == 0), stop=(kb == KB - 1),
                )
            q16 = stmp.tile([128, 512], f16, tag="t0")
            nc.vector.tensor_scalar_add(q16[:, :], ps_q, b_t[:, f:f + 1])
            # rope partner rows via SBUF->SBUF partition-swap DMAs (replaces
            # a PE perm matmul); split across both idle queues for latency
            qp = stmp.tile([128, 512], f16, tag="t3")
            nc.sync.dma_start(out=qp[0:32, :], in_=q16[32:64, :])
            nc.gpsimd.dma_start(out=qp[32:64, :], in_=q16[0:32, :])
            nc.sync.dma_start(out=qp[64:96, :], in_=q16[96:128, :])
            nc.gpsimd.dma_start(out=qp[96:128, :], in_=q16[64:96, :])
            flush_pend()
            st["pend"] = (qp, q16, dst, f, cs)

        def v_block(tcx, tl, pool=None):
            tt = tcx * 4 + tl
            x_t = x_ts[tcx]
            if pool is None:
                ps2 = ppq.tile([128, 1024], f32, tag="qq")
            else:
                ps2 = pool.tile([128, 2, 512], f32, tag="mm", name="ps2b").rearrange(
                    "p a b -> p (a b)")
            ps_v = ps2[:, 0:512]
            for kb in range(KB):
                nc.tensor.matmul(
                    ps_v,
                    x_t[:, kb, tl * 128:(tl + 1) * 128],
                    wv_t[:, kb, :],
                    start=(kb == 0), stop=(kb == KB - 1),
                )
            nc.vector.tensor_add(
                v_t[:, tt, :, D:128],
                ps_v.rearrange("p (h d) -> p h d", h=HL),
                bvb_t.rearrange("p (h d) -> p h d", h=HL),
            )

        def qkv_blocks(tcx, alternate=False):
            blocks = ([lambda f=f, p=None: qk_block(tcx, "q", f, p)
                       for f in range(4)]
                      + [lambda f=f, p=None: qk_block(tcx, "k", f, p)
                         for f in range(4)]
                      + [lambda tl=tl, p=None: v_block(tcx, tl, p)
                         for tl in range(4)])
            if alternate:
                # odd blocks borrow the (still idle) attention S pool so the
                # PE never serializes on the single qkv PSUM buffer
                return [lambda b=b, i=i: b(p=pps if i % 2 else None)
                        for i, b in enumerate(blocks)]
            return blocks

        def proj_tile(tt, pool=None):
            flush_pend()
            if pool is None:
                o_ps = ppq.tile([128, 1024], f32, tag="qq")
            else:
                o_ps = pool.tile([128, 2, 512], f32, tag="mm", name="o_psb").rearrange(
                    "p a b -> p (a b)")
            # two accumulation chains, each within one PSUM bank
            for half in range(2):
                for kb in range(4):
                    nc.tensor.matmul(
                        o_ps[:, half * 512:(half + 1) * 512],
                        ot_t[:, kb, tt * 128:(tt + 1) * 128],
                        wp_t[:, kb, half * 512:(half + 1) * 512],
                        start=(kb == 0), stop=(kb == 3),
                    )
            o_st = sout.tile([128, E], f16, tag="o")
            ts_ = slice(tt * 128, (tt + 1) * 128)
            for half, eng in ((0, nc.sync), (1, nc.gpsimd)):
                hs = slice(half * 512, (half + 1) * 512)
                nc.vector.tensor_add(o_st[:, hs], o_ps[:, hs], bpb_t[:, hs])
                eng.dma_start(out=out_ext[ts_, hs], in_=o_st[:, hs])

        # ---- attention
        ps_os = {}

        def emit_S(gi, hp, kt):
            q0 = gi * 512
            kt0 = gi * 4
            j = kt - kt0
            w0 = max(j, 0) * 128
            ncol = 512 - w0
            ps_s = pps.tile([128, 2, 512], f32, tag="mm")
            # one PSUM bank per head-half; exp uses a 2D AP over just the
            # valid cols of both halves (saves ~10% ACT on diagonal steps)
            for half, bp in ((0, 0), (1, 64)):
                nc.tensor.matmul(
                    ps_s[:, half, 0:ncol],
                    kT_t[bp:bp + 64, hp, kt * 128:(kt + 1) * 128],
                    qT_t[bp:bp + 64, hp, q0 + w0:q0 + 512],
                    start=True, stop=True,
                )
            return ps_s, ncol

        def emit_tail(gi, hp, kt, ps_s, ncol):
            q0 = gi * 512
            kt0 = gi * 4
            nkt = kt0 + 4
            j = kt - kt0
            w0 = 512 - ncol
            if kt == 0:
                ps_os[hp] = (pac.tile([128, 512], f32, tag="acc", name="ps_oA"),
                             pac.tile([128, 512], f32, tag="acc", name="ps_oB"))
            p_t = sp.tile([128, 2, 512], f16, tag="p")
            nc.scalar.activation(
                p_t[:, :, 0:ncol], ps_s[:, :, 0:ncol],
                mybir.ActivationFunctionType.Exp, scale=float(D) ** -0.5,
            )
            if j >= 0:
                nc.vector.tensor_mul(p_t[:, 0, 0:128], p_t[:, 0, 0:128],
                                     tri_t[:, :])
                nc.vector.tensor_mul(p_t[:, 1, 0:128], p_t[:, 1, 0:128],
                                     tri_t[:, :])
            for half, h, ps_o in ((0, 2 * hp, ps_os[hp][0]),
                                  (1, 2 * hp + 1, ps_os[hp][1])):
                nc.tensor.matmul(
                    ps_o[:, w0:512],
                    v_t[:, kt, h, :],
                    p_t[:, half, 0:ncol],
                    start=(kt == 0), stop=(kt == nkt - 1),
                )
            if kt == nkt - 1:
                qs = slice(q0, q0 + 512)
                for bp, ps_o in ((0, ps_os[hp][0]), (64, ps_os[hp][1])):
                    recip = ssm.tile([128, 512], f32, tag="rc")
                    nc.vector.reciprocal_approx_fast(out=recip[0:64, 0:512],
                                                     in_=ps_o[0:64, 0:512])
                    nc.vector.tensor_mul(ot_t[bp:bp + 64, hp, qs],
                                         ps_o[64:128, 0:512], recip[0:64, 0:512])
                del ps_os[hp]

        def attn_group(gi, fillers):
            nkt = gi * 4 + 4
            steps = [(hp, kt) for hp in range(HL // 2) for kt in range(nkt)]
            ns = len(steps)
            nf = len(fillers)
            # S emitted one step ahead so the in-order PE always has the
            # next pair's scores in flight while ACT runs the current exp.
            ps_prev = emit_S(gi, *steps[0])
            fi = 0
            for i, (hp, kt) in enumerate(steps):
                ps_cur = ps_prev
                if i + 1 < ns:
                    ps_prev = emit_S(gi, *steps[i + 1])
                emit_tail(gi, hp, kt, *ps_cur)
                # near an hp boundary, hold fillers back so the DVE can run
                # the accumulator-freeing recip/mul chain without backlog
                nkt_ = gi * 4 + 4
                if kt >= nkt_ - 2 and i + 1 < ns:
                    continue
                want = (i + 1) * nf // ns
                while fi < want:
                    fillers[fi]()
                    fi += 1
            while fi < nf:
                fillers[fi]()
                fi += 1

        # ---- emission schedule
        for blk in qkv_blocks(0, alternate=True):
            blk()
        flush_pend()
        attn_group(0, [lambda: dma_x(2)] + qkv_blocks(1))
        flush_pend()
        attn_group(1, [lambda: dma_x(3)] + qkv_blocks(2)
                   + [lambda t=t: proj_tile(t) for t in range(0, 4)])
        flush_pend()
        attn_group(2, qkv_blocks(3))
        flush_pend()
        attn_group(3, [lambda t=t: proj_tile(t) for t in range(4, 12)])
        flush_pend()
        for t in range(12, 16):
            proj_tile(t, pool=pps if t % 2 else None)

    nc.compile()
    return nc


_NC = None


def _get_nc():
    global _NC
    if _NC is None:
        _NC = build_nc()
    return _NC


def _host_prep(x, Wqkv, bqkv, Wproj, bproj):
    """Build the 8 per-core input maps."""
    x = np.asarray(x, np.float32)
    Wqkv = np.asarray(Wqkv, np.float32)
    bqkv = np.asarray(bqkv, np.float32)
    Wproj = np.asarray(Wproj, np.float32)
    bproj = np.asarray(bproj, np.float32)

    perm_d = np.concatenate([np.arange(0, D, 2), np.arange(1, D, 2)])  # evens, odds

    # rope tables (feature-major; rows r: freq r%32, sign -/+ per 32-block)
    inv_freq = 1.0 / ROPE_THETA ** (np.arange(0, D, 2, dtype=np.float32) / D)
    freqs = np.arange(T, dtype=np.float32)[:, None] * inv_freq[None, :]  # (T, 32)
    cosf = np.cos(freqs).T.astype(np.float32)  # (32, T)
    sinf = np.sin(freqs).T.astype(np.float32)
    ctab = np.tile(cosf, (4, 1)).astype(np.float16)                 # (128, T)
    stab = np.concatenate([-sinf, sinf, -sinf, sinf], 0).astype(np.float16)

    tri = (np.arange(128)[:, None] <= np.arange(128)[None, :]).astype(np.float16)

    maps = []
    for c in range(8):
        b, g = c // 2, c % 2
        heads = np.arange(8 * g, 8 * g + 8)
        # permuted q/k columns, natural v columns
        qcols = (heads[:, None] * D + perm_d[None, :]).ravel()
        vcols = (heads[:, None] * D + np.arange(D)[None, :]).ravel()
        if g == 0:
            bp_full = np.tile(bproj.astype(np.float32), (128, 1))
        else:
            bp_full = np.zeros((128, E), np.float32)
        def relay(w, kb):  # [kb*128, N] -> [128, kb, N] partition-major
            n = w.shape[1]
            return np.ascontiguousarray(
                w.reshape(kb, 128, n).transpose(1, 0, 2).astype(np.float16))

        def relay_f(w):  # [KB*128, 4*128] -> [128, 4, KB, 128] f-strip major
            return np.ascontiguousarray(
                w.reshape(KB, 128, 4, 128).transpose(1, 2, 0, 3)
                .astype(np.float16))

        x3 = np.ascontiguousarray(
            x[b].reshape(TC, 512, KB, 128).transpose(3, 0, 2, 1)
            .astype(np.float16))
        maps.append({
            "xT": x3,
            "wq": relay_f(Wqkv[:, qcols]),
            "wk": relay_f(Wqkv[:, E + qcols]),
            "wv": relay(Wqkv[:, 2 * E + vcols], KB),
            "bq": np.ascontiguousarray(bqkv[qcols].reshape(4, 128).T.astype(np.float32)),
            "bk": np.ascontiguousarray(bqkv[E + qcols].reshape(4, 128).T.astype(np.float32)),
            "bvb": np.ascontiguousarray(
                np.tile(bqkv[2 * E + vcols].astype(np.float16), (128, 1))),
            "wproj": relay(Wproj[vcols, :], 4),
            "bpb": bp_full,
            "ctab": ctab,
            "stab": stab,
            "tri": tri,
        })
    return maps


def kernel(x, Wqkv, bqkv, Wproj, bproj):
    nc = _get_nc()
    in_maps = _host_prep(x, Wqkv, bqkv, Wproj, bproj)
    res = run_bass_kernel_spmd(nc, in_maps, list(range(8)))
    out = np.empty((B, T, E), np.float32)
    for b in range(B):
        out[b] = (res.results[2 * b]["out"].astype(np.float32)
                  + res.results[2 * b + 1]["out"].astype(np.float32))
    return out


if __name__ == "__main__":
    rng = np.random.default_rng(0)
    x = rng.standard_normal((B, T, E), dtype=np.float32)
    Wqkv = rng.standard_normal((E, 3 * E), dtype=np.float32) * 0.02
    bqkv = rng.standard_normal((3 * E,), dtype=np.float32) * 0.02
    Wproj = rng.standard_normal((E, E), dtype=np.float32) * 0.02
    bproj = rng.standard_normal((E,), dtype=np.float32) * 0.02
    o = kernel(x=x, Wqkv=Wqkv, bqkv=bqkv, Wproj=Wproj, bproj=bproj)
    print("out", o.shape, o.dtype, float(np.abs(o).max()))


# revision 23
# speedup vs baseline: 1.0651x; 1.0393x over previous
"""Causal self-attention (B=4, T=2048, E=1024, H=16, rope) on 8 trn2 NeuronCores.

Sharding: core c = 2*b + g handles batch b = c//2, head-group g = c%2
(8 of the 16 heads).  Each core:
  - projects its batch's x into q,k (feature-major, rope'd on chip) and v
    for its 8 heads (fp16 matmuls, fp32 accumulate),
  - runs causal attention entirely on-chip (S^T tiles as stationary
    operands, ones-augmented v gives softmax denominators for free),
  - applies a PARTIAL output projection using only its local 512 inner
    dims (Wproj row-shard) -> no collective at all; the host sums the
    two partial [T, E] outputs of each (g=0, g=1) pair.

Scheduling: a single interleaved emission stream.  Attention is
ACT(exp)-bound (~1.1us/step) while its own PE work is ~0.8us/step, so
the next chunk's qkv projection blocks and the previous group's output
projection tiles are woven between attention steps as PE fillers,
keeping the (in-order) PE queue dense.
"""
import sys

for _p in ("/opt/trn_rl_repo", "/root/.axon_site/_ro/trn_rl_repo"):
    if _p not in sys.path:
        sys.path.append(_p)

import numpy as np
from contextlib import ExitStack

import concourse.bass as bass
import concourse.tile as tile
from concourse import bacc, mybir
from concourse.bass_utils import run_bass_kernel_spmd

B, T, E = 4, 2048, 1024
H_TOT, D = 16, 64
HL = 8            # heads per core
F = HL * D        # 512 local q/k/v features
KB = E // 128     # 8 contraction blocks for qkv
TC = T // 512     # 4 time chunks (512 cols)
TT = T // 128     # 16 time tiles
ROPE_THETA = 10000.0

f32 = mybir.dt.float32
f16 = mybir.dt.float16


def build_nc():
    nc = bacc.Bacc(None, target_bir_lowering=False, debug=False)

    xT = nc.declare_dram_parameter("xT", [128, TC, KB, 512], f16, isOutput=False)
    wq = nc.declare_dram_parameter("wq", [128, 4, KB, 128], f16, isOutput=False)
    wk = nc.declare_dram_parameter("wk", [128, 4, KB, 128], f16, isOutput=False)
    wv = nc.declare_dram_parameter("wv", [128, KB, F], f16, isOutput=False)
    bq = nc.declare_dram_parameter("bq", [128, 4], f32, isOutput=False)  # feature-major cols
    bk = nc.declare_dram_parameter("bk", [128, 4], f32, isOutput=False)
    bvb = nc.declare_dram_parameter("bvb", [128, F], f16, isOutput=False)
    wproj = nc.declare_dram_parameter("wproj", [128, 4, E], f16, isOutput=False)
    bpb = nc.declare_dram_parameter("bpb", [128, E], f32, isOutput=False)
    ctab_d = nc.declare_dram_parameter("ctab", [128, T], f16, isOutput=False)
    stab_d = nc.declare_dram_parameter("stab", [128, T], f16, isOutput=False)
    perm_d = nc.declare_dram_parameter("perm", [128, 128], f16, isOutput=False)
    tri_d = nc.declare_dram_parameter("tri", [128, 128], f16, isOutput=False)  # 0/1 mult mask
    out_ext = nc.declare_dram_parameter("out", [T, E], f16, isOutput=True)

    with ExitStack() as ctx:
        tc = ctx.enter_context(tile.TileContext(nc))
        sres = ctx.enter_context(tc.tile_pool(name="res", bufs=1))
        swts = ctx.enter_context(tc.tile_pool(name="wts", bufs=1))
        stab = ctx.enter_context(tc.tile_pool(name="tab", bufs=1))
        sx = ctx.enter_context(tc.tile_pool(name="x", bufs=2))
        stmp = ctx.enter_context(tc.tile_pool(name="tmp", bufs=2))
        sp = ctx.enter_context(tc.tile_pool(name="p", bufs=3))
        ssm = ctx.enter_context(tc.tile_pool(name="sm", bufs=2))
        sout = ctx.enter_context(tc.tile_pool(name="out", bufs=2))
        pps = ctx.enter_context(tc.tile_pool(name="ps", bufs=2, space="PSUM"))
        ppq = ctx.enter_context(tc.tile_pool(name="pq", bufs=1, space="PSUM"))
        pac = ctx.enter_context(tc.tile_pool(name="ac", bufs=2, space="PSUM"))

        # ---- resident tiles
        qT_t = sres.tile([128, 4, T], f16, tag="qT")       # rope'd q, feature-major
        kT_t = sres.tile([128, 4, T], f16, tag="kT")
        v_t = sres.tile([128, TT, HL, 128], f16, tag="v")  # ones 0:64, v 64:128
        ot_t = sres.tile([128, 4, T], f16, tag="ot")       # attention out, feature-major
        perm_t = sres.tile([128, 128], f16, tag="perm")
        tri_t = sres.tile([128, 128], f16, tag="tri")
        bq_t = sres.tile([128, 4], f32, tag="bq")
        bk_t = sres.tile([128, 4], f32, tag="bk")
        bvb_t = sres.tile([128, F], f16, tag="bvb")
        bpb_t = sres.tile([128, E], f32, tag="bpb")
        ctab_t = stab.tile([128, T], f16, tag="ct")
        stab_t = stab.tile([128, T], f16, tag="st")
        wq_t = swts.tile([128, 4, KB, 128], f16, tag="wq")
        wk_t = swts.tile([128, 4, KB, 128], f16, tag="wk")
        wv_t = swts.tile([128, KB, F], f16, tag="wv")
        wp_t = swts.tile([128, 4, E], f16, tag="wp")

        # ---- startup DMAs: inputs are host-laid-out so each partition is
        # one contiguous run (big DMA packets).  Each queue sustains only
        # ~125GB/s and transfers start ~8us in, so the first-needed data
        # (x chunk 0, wq f-strips) is split 3 ways / strip-wise and
        # priority-ordered so the first matmul can issue ~12us.
        x_ts = {}
        x_ts[0] = sx.tile([128, KB, 512], f16, tag="x", name="x0")
        nc.sync.dma_start(out=x_ts[0][:, 0:3, :], in_=xT[:, 0, 0:3, :])
        nc.scalar.dma_start(out=x_ts[0][:, 3:6, :], in_=xT[:, 0, 3:6, :])
        nc.gpsimd.dma_start(out=wq_t[:, 0], in_=wq[:, 0])
        nc.gpsimd.dma_start(out=x_ts[0][:, 6:8, :], in_=xT[:, 0, 6:8, :])
        nc.scalar.dma_start(out=wq_t[:, 1], in_=wq[:, 1])
        nc.gpsimd.dma_start(out=wq_t[:, 2], in_=wq[:, 2])
        nc.scalar.dma_start(out=wq_t[:, 3], in_=wq[:, 3])
        nc.gpsimd.dma_start(out=perm_t, in_=perm_d[:, :])
        nc.gpsimd.dma_start(out=bq_t, in_=bq[:, :])
        nc.gpsimd.dma_start(out=bk_t, in_=bk[:, :])
        nc.sync.dma_start(out=ctab_t, in_=ctab_d[:, :])
        nc.sync.dma_start(out=stab_t, in_=stab_d[:, :])
        nc.gpsimd.dma_start(out=wk_t[:, 0], in_=wk[:, 0])
        nc.scalar.dma_start(out=wk_t[:, 1], in_=wk[:, 1])
        nc.gpsimd.dma_start(out=wk_t[:, 2], in_=wk[:, 2])
        nc.scalar.dma_start(out=wk_t[:, 3], in_=wk[:, 3])
        nc.gpsimd.dma_start(out=wv_t[:, 0:4, :], in_=wv[:, 0:4, :])
        nc.scalar.dma_start(out=wv_t[:, 4:8, :], in_=wv[:, 4:8, :])
        nc.gpsimd.dma_start(out=bvb_t, in_=bvb[:, :])
        nc.gpsimd.dma_start(out=tri_t, in_=tri_d[:, :])
        # ones columns for the softmax-denominator trick, on gpsimd so the
        # DVE never waits; chunk-0 quarter first
        nc.gpsimd.memset(v_t[:, 0:4, :, 0:D], 1.0)
        nc.gpsimd.memset(v_t[:, 4:8, :, 0:D], 1.0)
        nc.gpsimd.dma_start(out=wp_t[:, 0:2, :], in_=wproj[:, 0:2, :])
        nc.scalar.dma_start(out=wp_t[:, 2:4, :], in_=wproj[:, 2:4, :])
        nc.gpsimd.dma_start(out=bpb_t, in_=bpb[:, :])
        nc.gpsimd.memset(v_t[:, 8:12, :, 0:D], 1.0)
        nc.gpsimd.memset(v_t[:, 12:16, :, 0:D], 1.0)

        def dma_x(tcx, eng2=None):
            eng2 = eng2 or nc.gpsimd
            x_ts[tcx] = sx.tile([128, KB, 512], f16, tag="x", name=f"x{tcx}")
            nc.sync.dma_start(out=x_ts[tcx][:, 0:4, :], in_=xT[:, tcx, 0:4, :])
            eng2.dma_start(out=x_ts[tcx][:, 4:8, :], in_=xT[:, tcx, 4:8, :])

        dma_x(1, eng2=nc.sync)

        # ---- qkv machinery (software-pipelined rope flush)
        st = {"pend": None}

        def flush_pend():
            if st["pend"] is None:
                return
            qp, ps_p, q16, dst, f, cs = st["pend"]
            st["pend"] = None
            if ps_p is not None:
                # startup path: partner rows via PE perm matmul (the DMA
                # queues are saturated with weight loads at this point)
                nc.tensor.matmul(ps_p, perm_t[:, :], q16[:, :],
                                 start=True, stop=True)
                qp = ps_p
            t1 = stmp.tile([128, 512], f16, tag="t1")
            nc.vector.tensor_mul(t1[:, :], q16[:, :], ctab_t[:, cs])
            t2 = stmp.tile([128, 512], f16, tag="t2")
            nc.vector.tensor_mul(t2[:, :], qp[:, :], stab_t[:, cs])
            nc.vector.tensor_add(dst[:, f, cs], t1[:, :], t2[:, :])

        def qk_block(tcx, which, f, pool=None):
            cs = slice(tcx * 512, (tcx + 1) * 512)
            w_t, b_t, dst = ((wq_t, bq_t, qT_t) if which == "q"
                             else (wk_t, bk_t, kT_t))
            x_t = x_ts[tcx]
            if pool is None:
                ps2 = ppq.tile([128, 1024], f32, tag="qq")
            else:
                ps2 = pool.tile([128, 2, 512], f32, tag="mm", name="ps2b").rearrange(
                    "p a b -> p (a b)")
            ps_q = ps2[:, 0:512]
            for kb in range(KB):
                nc.tensor.matmul(
                    ps_q,
                    w_t[:, f, kb, :],
                    x_t[:, kb, :],
                    start=(kb == 0), stop=(kb == KB - 1),
                )
            q16 = stmp.tile([128, 512], f16, tag="t0")
            nc.vector.tensor_scalar_add(q16[:, :], ps_q, b_t[:, f:f + 1])
            if tcx == 0:
                # startup: PE perm matmul (queues busy with weight loads)
                flush_pend()
                st["pend"] = (None, ps2[:, 512:1024], q16, dst, f, cs)
            else:
                # steady state: partner rows via SBUF->SBUF partition-swap
                # DMAs on the two bulk queues (saves PE + makes the rope
                # muls all-fp16 for DVE 2x)
                qp = stmp.tile([128, 512], f16, tag="t3")
                nc.sync.dma_start(out=qp[0:32, :], in_=q16[32:64, :])
                nc.gpsimd.dma_start(out=qp[32:64, :], in_=q16[0:32, :])
                nc.sync.dma_start(out=qp[64:96, :], in_=q16[96:128, :])
                nc.gpsimd.dma_start(out=qp[96:128, :], in_=q16[64:96, :])
                flush_pend()
                st["pend"] = (qp, None, q16, dst, f, cs)

        def v_block(tcx, tl, pool=None):
            tt = tcx * 4 + tl
            x_t = x_ts[tcx]
            if pool is None:
                ps2 = ppq.tile([128, 1024], f32, tag="qq")
            else:
                ps2 = pool.tile([128, 2, 512], f32, tag="mm", name="ps2b").rearrange(
                    "p a b -> p (a b)")
            ps_v = ps2[:, 0:512]
            for kb in range(KB):
                nc.tensor.matmul(
                    ps_v,
                    x_t[:, kb, tl * 128:(tl + 1) * 128],
                    wv_t[:, kb, :],
                    start=(kb == 0), stop=(kb == KB - 1),
                )
            nc.vector.tensor_add(
                v_t[:, tt, :, D:128],
                ps_v.rearrange("p (h d) -> p h d", h=HL),
                bvb_t.rearrange("p (h d) -> p h d", h=HL),
            )

        def qkv_blocks(tcx, alternate=False):
            blocks = ([lambda f=f, p=None: qk_block(tcx, "q", f, p)
                       for f in range(4)]
                      + [lambda f=f, p=None: qk_block(tcx, "k", f, p)
                         for f in range(4)]
                      + [lambda tl=tl, p=None: v_block(tcx, tl, p)
                         for tl in range(4)])
            if alternate:
                # odd blocks borrow the (still idle) attention S pool so the
                # PE never serializes on the single qkv PSUM buffer
                return [lambda b=b, i=i: b(p=pps if i % 2 else None)
                        for i, b in enumerate(blocks)]
            return blocks

        def proj_tile(tt, pool=None):
            flush_pend()
            if pool is None:
                o_ps = ppq.tile([128, 1024], f32, tag="qq")
            else:
                o_ps = pool.tile([128, 2, 512], f32, tag="mm", name="o_psb").rearrange(
                    "p a b -> p (a b)")
            # two accumulation chains, each within one PSUM bank
            for half in range(2):
                for kb in range(4):
                    nc.tensor.matmul(
                        o_ps[:, half * 512:(half + 1) * 512],
                        ot_t[:, kb, tt * 128:(tt + 1) * 128],
                        wp_t[:, kb, half * 512:(half + 1) * 512],
                        start=(kb == 0), stop=(kb == 3),
                    )
            o_st = sout.tile([128, E], f16, tag="o")
            ts_ = slice(tt * 128, (tt + 1) * 128)
            eng2 = nc.scalar if tt >= 12 else nc.gpsimd
            for half, eng in ((0, nc.sync), (1, eng2)):
                hs = slice(half * 512, (half + 1) * 512)
                nc.vector.tensor_add(o_st[:, hs], o_ps[:, hs], bpb_t[:, hs])
                eng.dma_start(out=out_ext[ts_, hs], in_=o_st[:, hs])

        # ---- attention
        ps_os = {}

        def emit_S(gi, hp, kt):
            q0 = gi * 512
            kt0 = gi * 4
            j = kt - kt0
            w0 = max(j, 0) * 128
            ncol = 512 - w0
            ps_s = pps.tile([128, 2, 512], f32, tag="mm")
            # one PSUM bank per head-half; exp uses a 2D AP over just the
            # valid cols of both halves (saves ~10% ACT on diagonal steps)
            for half, bp in ((0, 0), (1, 64)):
                nc.tensor.matmul(
                    ps_s[:, half, 0:ncol],
                    kT_t[bp:bp + 64, hp, kt * 128:(kt + 1) * 128],
                    qT_t[bp:bp + 64, hp, q0 + w0:q0 + 512],
                    start=True, stop=True,
                )
            return ps_s, ncol

        def emit_tail(gi, hp, kt, ps_s, ncol):
            q0 = gi * 512
            kt0 = gi * 4
            nkt = kt0 + 4
            j = kt - kt0
            w0 = 512 - ncol
            if kt == 0:
                ps_os[hp] = (pac.tile([128, 512], f32, tag="acc", name="ps_oA"),
                             pac.tile([128, 512], f32, tag="acc", name="ps_oB"))
            p_t = sp.tile([128, 2, 512], f16, tag="p")
            nc.scalar.activation(
                p_t[:, :, 0:ncol], ps_s[:, :, 0:ncol],
                mybir.ActivationFunctionType.Exp, scale=float(D) ** -0.5,
            )
            if j >= 0:
                nc.vector.tensor_mul(p_t[:, 0, 0:128], p_t[:, 0, 0:128],
                                     tri_t[:, :])
                nc.vector.tensor_mul(p_t[:, 1, 0:128], p_t[:, 1, 0:128],
                                     tri_t[:, :])
            for half, h, ps_o in ((0, 2 * hp, ps_os[hp][0]),
                                  (1, 2 * hp + 1, ps_os[hp][1])):
                nc.tensor.matmul(
                    ps_o[:, w0:512],
                    v_t[:, kt, h, :],
                    p_t[:, half, 0:ncol],
                    start=(kt == 0), stop=(kt == nkt - 1),
                )
            if kt == nkt - 1:
                qs = slice(q0, q0 + 512)
                for bp, ps_o in ((0, ps_os[hp][0]), (64, ps_os[hp][1])):
                    recip = ssm.tile([128, 512], f32, tag="rc")
                    nc.vector.reciprocal_approx_fast(out=recip[0:64, 0:512],
                                                     in_=ps_o[0:64, 0:512])
                    nc.vector.tensor_mul(ot_t[bp:bp + 64, hp, qs],
                                         ps_o[64:128, 0:512], recip[0:64, 0:512])
                del ps_os[hp]

        def attn_group(gi, fillers):
            nkt = gi * 4 + 4
            steps = [(hp, kt) for hp in range(HL // 2) for kt in range(nkt)]
            ns = len(steps)
            nf = len(fillers)
            # S emitted one step ahead so the in-order PE always has the
            # next pair's scores in flight while ACT runs the current exp.
            ps_prev = emit_S(gi, *steps[0])
            fi = 0
            for i, (hp, kt) in enumerate(steps):
                ps_cur = ps_prev
                if i + 1 < ns:
                    ps_prev = emit_S(gi, *steps[i + 1])
                emit_tail(gi, hp, kt, *ps_cur)
                # near an hp boundary, hold fillers back so the DVE can run
                # the accumulator-freeing recip/mul chain without backlog
                nkt_ = gi * 4 + 4
                if kt >= nkt_ - 2 and i + 1 < ns:
                    continue
                want = (i + 1) * nf // ns
                while fi < want:
                    fillers[fi]()
                    fi += 1
            while fi < nf:
                fillers[fi]()
                fi += 1

        # ---- emission schedule
        for blk in qkv_blocks(0, alternate=True):
            blk()
        flush_pend()
        attn_group(0, [lambda: dma_x(2)] + qkv_blocks(1))
        flush_pend()
        attn_group(1, [lambda: dma_x(3)] + qkv_blocks(2)
                   + [lambda t=t: proj_tile(t) for t in range(0, 4)])
        flush_pend()
        attn_group(2, qkv_blocks(3))
        flush_pend()
        attn_group(3, [lambda t=t: proj_tile(t) for t in range(4, 12)])
        flush_pend()
        for t in range(12, 16):
            proj_tile(t, pool=pps if t % 2 else None)

    nc.compile()
    return nc


_NC = None


def _get_nc():
    global _NC
    if _NC is None:
        _NC = build_nc()
    return _NC


def _host_prep(x, Wqkv, bqkv, Wproj, bproj):
    """Build the 8 per-core input maps."""
    x = np.asarray(x, np.float32)
    Wqkv = np.asarray(Wqkv, np.float32)
    bqkv = np.asarray(bqkv, np.float32)
    Wproj = np.asarray(Wproj, np.float32)
    bproj = np.asarray(bproj, np.float32)

    perm_d = np.concatenate([np.arange(0, D, 2), np.arange(1, D, 2)])  # evens, odds

    # rope tables (feature-major; rows r: freq r%32, sign -/+ per 32-block)
    inv_freq = 1.0 / ROPE_THETA ** (np.arange(0, D, 2, dtype=np.float32) / D)
    freqs = np.arange(T, dtype=np.float32)[:, None] * inv_freq[None, :]  # (T, 32)
    cosf = np.cos(freqs).T.astype(np.float32)  # (32, T)
    sinf = np.sin(freqs).T.astype(np.float32)
    ctab = np.tile(cosf, (4, 1)).astype(np.float16)                 # (128, T)
    stab = np.concatenate([-sinf, sinf, -sinf, sinf], 0).astype(np.float16)

    # block-swap permutation matrix: out row m <- in row pi(m)
    pmat = np.zeros((128, 128), np.float16)
    for m in range(128):
        base = (m // 64) * 64
        r = m % 64
        pmat[base + (r + 32) % 64, m] = 1.0

    tri = (np.arange(128)[:, None] <= np.arange(128)[None, :]).astype(np.float16)

    maps = []
    for c in range(8):
        b, g = c // 2, c % 2
        heads = np.arange(8 * g, 8 * g + 8)
        # permuted q/k columns, natural v columns
        qcols = (heads[:, None] * D + perm_d[None, :]).ravel()
        vcols = (heads[:, None] * D + np.arange(D)[None, :]).ravel()
        if g == 0:
            bp_full = np.tile(bproj.astype(np.float32), (128, 1))
        else:
            bp_full = np.zeros((128, E), np.float32)
        def relay(w, kb):  # [kb*128, N] -> [128, kb, N] partition-major
            n = w.shape[1]
            return np.ascontiguousarray(
                w.reshape(kb, 128, n).transpose(1, 0, 2).astype(np.float16))

        def relay_f(w):  # [KB*128, 4*128] -> [128, 4, KB, 128] f-strip major
            return np.ascontiguousarray(
                w.reshape(KB, 128, 4, 128).transpose(1, 2, 0, 3)
                .astype(np.float16))

        x3 = np.ascontiguousarray(
            x[b].reshape(TC, 512, KB, 128).transpose(3, 0, 2, 1)
            .astype(np.float16))
        maps.append({
            "xT": x3,
            "wq": relay_f(Wqkv[:, qcols]),
            "wk": relay_f(Wqkv[:, E + qcols]),
            "wv": relay(Wqkv[:, 2 * E + vcols], KB),
            "bq": np.ascontiguousarray(bqkv[qcols].reshape(4, 128).T.astype(np.float32)),
            "bk": np.ascontiguousarray(bqkv[E + qcols].reshape(4, 128).T.astype(np.float32)),
            "bvb": np.ascontiguousarray(
                np.tile(bqkv[2 * E + vcols].astype(np.float16), (128, 1))),
            "wproj": relay(Wproj[vcols, :], 4),
            "bpb": bp_full,
            "ctab": ctab,
            "stab": stab,
            "perm": pmat,
            "tri": tri,
        })
    return maps


def kernel(x, Wqkv, bqkv, Wproj, bproj):
    nc = _get_nc()
    in_maps = _host_prep(x, Wqkv, bqkv, Wproj, bproj)
    res = run_bass_kernel_spmd(nc, in_maps, list(range(8)))
    out = np.empty((B, T, E), np.float32)
    for b in range(B):
        out[b] = (res.results[2 * b]["out"].astype(np.float32)
                  + res.results[2 * b + 1]["out"].astype(np.float32))
    return out


if __name__ == "__main__":
    rng = np.random.default_rng(0)
    x = rng.standard_normal((B, T, E), dtype=np.float32)
    Wqkv = rng.standard_normal((E, 3 * E), dtype=np.float32) * 0.02
    bqkv = rng.standard_normal((3 * E,), dtype=np.float32) * 0.02
    Wproj = rng.standard_normal((E, E), dtype=np.float32) * 0.02
    bproj = rng.standard_normal((E,), dtype=np.float32) * 0.02
    o = kernel(x=x, Wqkv=Wqkv, bqkv=bqkv, Wproj=Wproj, bproj=bproj)
    print("out", o.shape, o.dtype, float(np.abs(o).max()))
